# revision 29
# baseline (speedup 1.0000x reference)
"""Trainium2 Bass kernel for an 8-layer weight-shared dense transformer variant.

Sharding: data-parallel over batch B=2 x tensor-parallel over the NH=4 heads
(core = 4*b + h). Each core runs the full layer stack for its (b, h) pair;
the yMLP head-contributions are summed with AllReduces over the 4 cores of
each batch group per layer.

v5 structure (per layer) - numerics identical to v2, pipelined collectives:
  - enc and encv are RESIDENT in SBUF bf16 (loaded once; the layer stack
    shares weights, so per-layer weight streaming was pure waste).  dec
    streams in batched [128,8,256] chunks per pass.
  - Phase D/E is split into two T-half passes; each half's yMLP is
    transpose-staged ([T,D] layout via PE transposes, so the collective
    needs no DMA transposes) and all-reduced separately.  CC(half0)
    overlaps the half1 compute pass; CC(half1) overlaps the next layer's
    first phase-A half-sweep, whose program order is interleaved with the
    previous layer's phase-F halves so no engine queue head-blocks.
  - Phase A runs as two t-half sweeps (matmul pair -> relu -> rope ->
    QRT half / spill half); the wide B1 score strips accumulate inside
    sweep 1 with a pair-group lag.  QR is fp8e4 x16, scores run DoubleRow
    fp8 trimmed to the causal triangle, rescaled 1/256 on the psum copy.
  - Narrow B2 strips + yKV tiles are interleaved so scores, LN chains and
    yKVt PE-transposes pipeline; phase D starts as soon as the first
    half of yKVt exists.
  - x_sparse spills to DRAM bf16 in [128,2,512] half DMAs and reloads
    pairwise per half in the D passes.
"""

import math
import os

import ml_dtypes
import numpy as np

import concourse.bass as bass
import concourse.mybir as mybir
import concourse.tile as tile
from concourse.bass_utils import run_bass_kernel_spmd
from concourse.masks import make_causal_mask, make_identity

BF16 = mybir.dt.bfloat16
F32 = mybir.dt.float32
FP8 = mybir.dt.float8e4
AF = mybir.ActivationFunctionType
ALU = mybir.AluOpType
DR = mybir.MatmulPerfMode.DoubleRow

N_CORES = 8
B, T, D, NH, N, VOCAB = 2, 1024, 256, 4, 8192, 256
NPAIR = N // 2
NTILES = N // 128  # 64
NGROUP = NPAIR // 128  # 32 pair groups (= packed k-tiles for DoubleRow)
TBLK = T // 128  # 8
TH = T // 2  # 512: the t-half width everything pipelines on
EPS = 1e-5
THETA = 2.0 ** 16
QSCALE = 16.0  # rope tables premultiplied by this; scores rescaled by 1/256
N_LAYER = int(os.environ.get("KERNEL_N_LAYER", "8"))
ABLATE_CC = os.environ.get("KERNEL_ABLATE_CC", "0") == "1"

REPLICA_GROUPS = [[0, 1, 2, 3], [4, 5, 6, 7]]

B1_CHUNKS = [(0, 0, 512), (0, 512, 512),
             (1, 128, 512), (1, 640, 384),
             (2, 256, 512)]
B2_CHUNKS = [(2, 768, 256),
             (3, 384, 512), (3, 896, 128),
             (4, 512, 512), (5, 640, 384),
             (6, 768, 256), (7, 896, 128)]
_SC_BASE = [0, 8, 15, 21, 26, 30, 33, 35]


def _sc_idx(sb, tb):
    return _SC_BASE[sb] + (tb - sb)


N_SC = 36


# ---------------------------------------------------------------- host side

def _split_multiwaits_json(bir: bytes) -> bytes:
    """This walrus build rejects instructions carrying more than one sync-wait
    ("Too many sync wait commands"), while Tile freely attaches several.
    Split: hoist all but the last wait of each instruction onto NoOps inserted
    immediately before it (same engine => executes right before it, so the
    AND-of-waits semantics is preserved)."""
    import json

    m = json.loads(bir)
    for func in m["functions"]:
        for blk in func["blocks"]:
            insts = blk["instructions"]
            out = []
            for inst in insts:
                si = inst.get("sync_info")
                waits = (si or {}).get("on_wait") or []
                if len(waits) > 1:
                    for k, w in enumerate(waits[:-1]):
                        out.append({
                            "engine": inst["engine"],
                            "ins": [],
                            "name": f"hw_{inst['name']}_{k}",
                            "opcode": "NoOp",
                            "outs": [],
                            "sync_info": {"on_update": [], "on_wait": [w]},
                        })
                    si["on_wait"] = [waits[-1]]
                out.append(inst)
            blk["instructions"] = out
    return json.dumps(m).encode()


def _install_json_fix(nc):
    orig = nc.to_json_bytes

    def patched():
        return _split_multiwaits_json(orig())

    nc.to_json_bytes = patched


def _pair_perm():
    """new index k -> original n.  Group j (256 original neurons) becomes an
    'even' tile (rows 2r) followed by an 'odd' tile (rows 2r+1)."""
    perm = np.empty(N, dtype=np.int64)
    r = np.arange(128)
    for j in range(NGROUP):
        base = 256 * j
        perm[base: base + 128] = base + 2 * r
        perm[base + 128: base + 256] = base + 2 * r + 1
    return perm


def _tables():
    """Rope table [NGROUP, 128, 2, T] bf16 = QSCALE * [c, -s] per pair row
    block.  With xsp2 = [e, o]:
      prods01 = xsp2 * tab        = [e*c, -o*s]  -> even tile = p0 + p1
      p2 = e * tab[1] = -e*s ; p3 = o * tab[0] = o*c -> odd = p3 - p2."""
    p = np.arange(NPAIR, dtype=np.float64)
    f = 1.0 / (THETA ** (2.0 * p / N)) / (2.0 * math.pi)
    t = np.arange(T, dtype=np.float64)
    ang = 2.0 * math.pi * np.mod(t[None, :] * f[:, None], 1.0)
    c = (QSCALE * np.cos(ang)).reshape(NGROUP, 128, T)
    s = (QSCALE * np.sin(ang)).reshape(NGROUP, 128, T)
    tab = np.stack([c, -s], axis=2)  # [NGROUP, 128, 2, T]
    return np.ascontiguousarray(tab).astype(ml_dtypes.bfloat16)


def _ln_np(x):
    x = x.astype(np.float64)
    mu = x.mean(-1, keepdims=True)
    var = ((x - mu) ** 2).mean(-1, keepdims=True)
    return ((x - mu) / np.sqrt(var + EPS)).astype(np.float32)


# ---------------------------------------------------------------- bass build

def build_nc(n_layers=N_LAYER):
    nc = bass.Bass("TRN2", target_bir_lowering=False, debug=False,
                   num_devices=N_CORES)

    x0_d = nc.dram_tensor("x0", [T, D], F32, kind="ExternalInput")
    enc_d = nc.dram_tensor("enc", [D, N], BF16, kind="ExternalInput")
    encv_d = nc.dram_tensor("encv", [D, N], BF16, kind="ExternalInput")
    dec_d = nc.dram_tensor("dec", [N, D], BF16, kind="ExternalInput")
    tab_d = nc.dram_tensor("ropetab", [NGROUP, 128, 2, T], BF16,
                           kind="ExternalInput")
    lmh_d = nc.dram_tensor("lmh", [D, VOCAB], BF16, kind="ExternalInput")
    out_d = nc.dram_tensor("logits", [T, VOCAB], F32, kind="ExternalOutput")
    xsp_d = nc.dram_tensor("xsp_scratch", [NTILES, 128, T], BF16).ap()
    cc_in_d = nc.dram_tensor("cc_in", [T, D], BF16).ap()
    cc_out_d = nc.dram_tensor("cc_out", [T, D], BF16).ap()
    warm_in = nc.dram_tensor("warm_in", [1, 64], BF16).ap()
    warm_out = nc.dram_tensor("warm_out", [1, 64], BF16).ap()

    enc_r = enc_d.ap().rearrange("(dh p) n -> p dh n", p=128)
    encv_r = encv_d.ap().rearrange("(dh p) n -> p dh n", p=128)
    dec_r = dec_d.ap().rearrange("(nt p) d -> p nt d", p=128)
    x0_r = x0_d.ap().rearrange("(i p) d -> p i d", p=128)
    lmh_r = lmh_d.ap().rearrange("(dh p) v -> p dh v", p=128)
    out_r = out_d.ap().rearrange("(i p) v -> p i v", p=128)
    tab_r = tab_d.ap()
    # batched spill view: pair pr covers tiles (2pr, 2pr+1) as [128, 2, T]
    xsp_pair = xsp_d.rearrange("(pr q) p t -> p pr q t", q=2)
    cc_in_r = cc_in_d.rearrange("(i p) d -> p i d", p=128)
    cc_out_r = cc_out_d.rearrange("(i p) d -> p i d", p=128)

    from contextlib import ExitStack

    with tile.TileContext(nc) as tc, ExitStack() as ctx:
        const = ctx.enter_context(tc.tile_pool(name="const", bufs=1))
        xpool = ctx.enter_context(tc.tile_pool(name="xpool", bufs=1))
        xbfp = ctx.enter_context(tc.tile_pool(name="xbfp", bufs=1))
        xtp = ctx.enter_context(tc.tile_pool(name="xtp", bufs=1))
        sthp = ctx.enter_context(tc.tile_pool(name="sthp", bufs=2))
        sttp = ctx.enter_context(tc.tile_pool(name="sttp", bufs=1))
        lnp = ctx.enter_context(tc.tile_pool(name="lnp", bufs=2))
        stats = ctx.enter_context(tc.tile_pool(name="stats", bufs=6))
        # persistent so next layer's tab loads overlap this layer's tail
        tabs = ctx.enter_context(tc.tile_pool(name="tabs", bufs=3))

        trim = const.tile([128, 128], F32, name="trim")
        make_causal_mask(nc, trim, mask_val=1.0)  # 1.0 where t > s else 0.0
        ident16 = const.tile([128, 128], BF16, name="ident16")
        make_identity(nc, ident16)
        lmh_t = const.tile([128, 2, VOCAB], BF16, name="lmh_t")
        nc.sync.dma_start(out=lmh_t, in_=lmh_r)
        eps_t = const.tile([128, 1], F32, name="eps_t")
        nc.vector.memset(eps_t, EPS)

        # resident bf16 weights, loaded once (shared across all layers);
        # loads are emitted after the x0 forms below and chunked so layer 0's
        # first matmuls don't wait on the full 8MB behind a serial DMA queue
        enc_s = const.tile([128, 2, N], BF16, name="enc_s")
        encv_s = const.tile([128, 2, N], BF16, name="encv_s")

        def layer_norm_tile(dst, src):
            """LN over the contiguous free (256) dim of one (128, 256) tile."""
            st = stats.tile([128, 6], F32, name="bn_st", tag="bn_st")
            mv = stats.tile([128, 2], F32, name="bn_mv", tag="bn_mv")
            rs = stats.tile([128, 1], F32, name="rstd", tag="rstd")
            nc.vector.bn_stats(out=st, in_=src)
            nc.vector.bn_aggr(out=mv, in_=st)
            nc.scalar.activation(out=rs, in_=mv[:, 1:2], func=AF.Sqrt,
                                 bias=eps_t, scale=1.0)
            nc.vector.reciprocal(rs, rs)
            nc.vector.tensor_scalar(
                out=dst, in0=src, scalar1=mv[:, 0:1], scalar2=rs,
                op0=ALU.subtract, op1=ALU.mult,
            )

        def make_x_forms(x_f32, tpp):
            """Initial x_bf [t,d] bf16 copy + xT [d,2,T] via PE transposes
            (DMA xbar transposes serialize against collectives in Tile)."""
            x_bf = xbfp.tile([128, TBLK, D], BF16, name="x_bf", tag="x_bf")
            xT = xtp.tile([128, 2, T], BF16, name="xT", tag="xT")
            for i in range(TBLK):
                nc.scalar.copy(out=x_bf[:, i, :], in_=x_f32[:, i, :])
                for dh in range(2):
                    tp = tpp.tile([128, 128], BF16, name="mxf", tag="mxf")
                    nc.tensor.transpose(
                        tp, x_bf[:, i, dh * 128:(dh + 1) * 128], ident16)
                    nc.scalar.copy(
                        out=xT[:, dh, i * 128:(i + 1) * 128], in_=tp)
            return x_bf, xT

        # warm up the collective path so layer 0's all-reduce doesn't pay
        # the one-time comm-init cost on the critical path
        if not ABLATE_CC:
            nc.gpsimd.collective_compute(
                "AllReduce", ALU.add, replica_groups=REPLICA_GROUPS,
                ins=[warm_in], outs=[warm_out])

        # initial x forms
        x_f32 = xpool.tile([128, TBLK, D], F32, name="x", tag="x")
        nc.sync.dma_start(out=x_f32, in_=x0_r)
        with tc.tile_pool(name="mxfp", bufs=2, space="PSUM") as mxfp:
            x_bf, xT = make_x_forms(x_f32, mxfp)
        for ch in range(4):
            nc.sync.dma_start(out=enc_s[:, :, ch * 2048:(ch + 1) * 2048],
                              in_=enc_r[:, :, ch * 2048:(ch + 1) * 2048])
        nc.sync.dma_start(out=encv_s, in_=encv_r)

        def emit_f_half(th, x_bf, xT, ftp, logits_pools=None, pin=False):
            """Phase F for t-half th of the PREVIOUS layer's cc output:
            LN + residual update (in place on x_f32) + x_bf/xT forms via
            PE transposes (DMA xbar transposes serialize against
            collectives in Tile, which would expose the CC latency).
            pin=True allocates yln from the tabs pool so its cc-gated DMA
            picks up a WAR dep on a late sweep-0 tab tile - otherwise the
            scheduler hoists it to the SP queue head, where its cc wait
            blocks every later DMA."""
            if pin:
                yln = tabs.tile([128, 4, D], BF16, name=f"yln{th}",
                                tag="tabh")
            else:
                yln = sthp.tile([128, 4, D], BF16, name=f"yln{th}",
                                tag="sth")
            nc.sync.dma_start(out=yln, in_=cc_out_r[:, th * 4:th * 4 + 4, :])
            for q in range(4):
                i = th * 4 + q
                ln_in = lnp.tile([128, D], F32, name="ln_in", tag="ln_in")
                layer_norm_tile(ln_in, yln[:, q, :])
                nc.vector.tensor_add(x_f32[:, i, :], x_f32[:, i, :], ln_in)
                layer_norm_tile(x_f32[:, i, :], x_f32[:, i, :])
                nc.scalar.copy(out=x_bf[:, i, :], in_=x_f32[:, i, :])
                for dh in range(2):
                    tp = ftp.tile([128, 128], BF16, name="fxt", tag="fxt")
                    nc.tensor.transpose(
                        tp, x_bf[:, i, dh * 128:(dh + 1) * 128], ident16)
                    nc.scalar.copy(
                        out=xT[:, dh, i * 128:(i + 1) * 128], in_=tp)
                if logits_pools is not None:
                    lps, lout = logits_pools
                    ps = lps.tile([128, VOCAB], F32, name="l_ps", tag="l_ps")
                    nc.tensor.matmul(
                        ps, lhsT=xT[:, 0, i * 128:(i + 1) * 128],
                        rhs=lmh_t[:, 0, :], start=True, stop=False)
                    nc.tensor.matmul(
                        ps, lhsT=xT[:, 1, i * 128:(i + 1) * 128],
                        rhs=lmh_t[:, 1, :], start=False, stop=True)
                    ot = lout.tile([128, VOCAB], F32, name="l_sb", tag="l_sb")
                    nc.scalar.copy(out=ot, in_=ps)
                    nc.sync.dma_start(out=out_r[:, i, :], in_=ot)

        for li in range(n_layers):
            if li > 0:
                x_bf = xbfp.tile([128, TBLK, D], BF16, name="x_bf",
                                 tag="x_bf")
                xT = xtp.tile([128, 2, T], BF16, name="xT", tag="xT")
            with ExitStack() as lctx:
                qrt_p = lctx.enter_context(tc.tile_pool(name="qrt", bufs=1))
                scp = lctx.enter_context(tc.tile_pool(name="scp", bufs=1))
                QRT = qrt_p.tile([128, NTILES, T], FP8, name="QRT", tag="QRT")
                scb = scp.tile([128, N_SC, 128], BF16, name="scb", tag="scb")

                # -------- phase A: two t-half sweeps (+ prev layer's F
                # halves interleaved so CC(half1) hides under sweep 0)
                with ExitStack() as actx:
                    xsps = actx.enter_context(tc.tile_pool(name="xsps",
                                                           bufs=3))
                    ropet = actx.enter_context(tc.tile_pool(name="ropet",
                                                            bufs=2))
                    apsum = actx.enter_context(
                        tc.tile_pool(name="apsum", bufs=2, space="PSUM"))
                    ftp = actx.enter_context(
                        tc.tile_pool(name="ftp", bufs=1, space="PSUM"))
                    b1psum = actx.enter_context(
                        tc.tile_pool(name="b1psum", bufs=1, space="PSUM"))
                    b1_ps = {
                        ch: b1psum.tile([128, ch[2]], F32, name=f"b1_{i}",
                                        tag=f"b1_{i}")
                        for i, ch in enumerate(B1_CHUNKS)
                    }
                    LAG = 2

                    def b1_step(kt):
                        for ch in B1_CHUNKS:
                            sb, t0, w = ch
                            nc.tensor.matmul(
                                b1_ps[ch],
                                lhsT=QRT[:, 2 * kt:2 * kt + 2,
                                         sb * 128:(sb + 1) * 128],
                                rhs=QRT[:, 2 * kt:2 * kt + 2, t0:t0 + w],
                                start=(kt == 0), stop=(kt == NGROUP - 1),
                                perf_mode=DR, skip_group_check=True)

                    def sweep(th):
                        tsl = slice(th * TH, (th + 1) * TH)
                        for j in range(NGROUP):
                            tabh = tabs.tile([128, 2, TH], BF16, name="tabh",
                                             tag="tabh")
                            nc.sync.dma_start(out=tabh,
                                              in_=tab_r[j][:, :, tsl])
                            xsp2 = xsps.tile([128, 2, TH], BF16, name="xsp2",
                                             tag="xsp2")
                            prods = ropet.tile([128, 4, TH], BF16,
                                               name="prods", tag="prods")
                            for par in range(2):
                                nseg = slice(256 * j + 128 * par,
                                             256 * j + 128 * par + 128)
                                ps = apsum.tile([128, TH], F32, name="aps",
                                                tag="aps")
                                nc.tensor.matmul(
                                    ps, lhsT=enc_s[:, 0, nseg],
                                    rhs=xT[:, 0, tsl],
                                    start=True, stop=False)
                                nc.tensor.matmul(
                                    ps, lhsT=enc_s[:, 1, nseg],
                                    rhs=xT[:, 1, tsl],
                                    start=False, stop=True)
                                nc.scalar.activation(
                                    out=xsp2[:, par, :], in_=ps,
                                    func=AF.Relu)
                                # prods = [e*c, -e*s, o*c, -o*s]
                                nc.vector.tensor_mul(
                                    prods[:, 2 * par, :], xsp2[:, par, :],
                                    tabh[:, 0, :])
                                nc.vector.tensor_mul(
                                    prods[:, 2 * par + 1, :],
                                    xsp2[:, par, :], tabh[:, 1, :])
                            nc.sync.dma_start(
                                out=xsp_pair[:, j][:, :, tsl], in_=xsp2)
                            # even = e*c + o*(-s); odd = o*c + e*s
                            nc.vector.tensor_add(QRT[:, 2 * j, tsl],
                                                 prods[:, 0, :],
                                                 prods[:, 3, :])
                            nc.gpsimd.tensor_sub(QRT[:, 2 * j + 1, tsl],
                                                 prods[:, 2, :],
                                                 prods[:, 1, :])
                            if th == 1 and j >= LAG:
                                b1_step(j - LAG)

                    if li > 0:
                        emit_f_half(0, x_bf, xT, ftp)
                    sweep(0)
                    if li > 0:
                        emit_f_half(1, x_bf, xT, ftp, pin=True)
                    sweep(1)
                    for kt in range(NGROUP - LAG, NGROUP):
                        b1_step(kt)
                    # copy B1 strips to sc blocks (scaled 1/256, diag trimmed)
                    for ch in B1_CHUNKS:
                        sb, t0, w = ch
                        for q in range(w // 128):
                            tb = t0 // 128 + q
                            dst = scb[:, _sc_idx(sb, tb), :]
                            src = b1_ps[ch][:, q * 128:(q + 1) * 128]
                            if tb == sb:
                                nc.vector.scalar_tensor_tensor(
                                    out=dst, in0=src, scalar=1.0 / 256.0,
                                    in1=trim, op0=ALU.mult, op1=ALU.mult)
                            else:
                                nc.scalar.activation(out=dst, in_=src,
                                                     func=AF.Copy,
                                                     scale=1.0 / 256.0)

                # ---------------- phase B2 + C interleaved
                with ExitStack() as bctx:
                    b2psum = bctx.enter_context(
                        tc.tile_pool(name="b2psum", bufs=2, space="PSUM"))
                    ykvln = lctx.enter_context(
                        tc.tile_pool(name="ykvln", bufs=1))
                    ykvq = lctx.enter_context(
                        tc.tile_pool(name="ykvq", bufs=2))
                    yKVt = ykvln.tile([128, 2, T], BF16, name="yKVt",
                                      tag="yKVt")
                    ykvps = bctx.enter_context(
                        tc.tile_pool(name="ykvps", bufs=2, space="PSUM"))
                    ctp = bctx.enter_context(
                        tc.tile_pool(name="ctp", bufs=2, space="PSUM"))

                    def b2_chunk(i):
                        sb, t0, w = B2_CHUNKS[i]
                        ps = b2psum.tile([128, w], F32, name=f"b2_{i}",
                                         tag="b2")
                        for j in range(NGROUP):
                            nc.tensor.matmul(
                                ps,
                                lhsT=QRT[:, 2 * j:2 * j + 2,
                                         sb * 128:(sb + 1) * 128],
                                rhs=QRT[:, 2 * j:2 * j + 2, t0:t0 + w],
                                start=(j == 0), stop=(j == NGROUP - 1),
                                perf_mode=DR)
                        for q in range(w // 128):
                            tb = t0 // 128 + q
                            dst = scb[:, _sc_idx(sb, tb), :]
                            srcp = ps[:, q * 128:(q + 1) * 128]
                            if tb == sb:
                                nc.vector.scalar_tensor_tensor(
                                    out=dst, in0=srcp, scalar=1.0 / 256.0,
                                    in1=trim, op0=ALU.mult, op1=ALU.mult)
                            else:
                                nc.scalar.activation(out=dst, in_=srcp,
                                                     func=AF.Copy,
                                                     scale=1.0 / 256.0)

                    def ykv_tile(tb):
                        yk = ykvps.tile([128, D], F32, name="yk", tag="yk")
                        for sb in range(tb + 1):
                            nc.tensor.matmul(
                                yk,
                                lhsT=scb[:, _sc_idx(sb, tb), :],
                                rhs=x_bf[:, sb, :],
                                start=(sb == 0), stop=(sb == tb))
                        ykv_td = ykvq.tile([128, D], BF16, name="ykv_td",
                                           tag="ykv_td")
                        layer_norm_tile(ykv_td, yk)
                        for dh in range(2):
                            tp = ctp.tile([128, 128], BF16, name="ctpt",
                                          tag="ctpt")
                            nc.tensor.transpose(
                                tp, ykv_td[:, dh * 128:(dh + 1) * 128],
                                ident16)
                            nc.scalar.copy(
                                out=yKVt[:, dh, tb * 128:(tb + 1) * 128],
                                in_=tp)

                    b2_after = {2: 1, 3: 3, 4: 4, 5: 5, 6: 6, 7: 7}
                    emitted = 0
                    for tb in range(TBLK):
                        while emitted < b2_after.get(tb, 0):
                            b2_chunk(emitted)
                            emitted += 1
                        ykv_tile(tb)
                    while emitted < len(B2_CHUNKS):
                        b2_chunk(emitted)
                        emitted += 1

                # -------- phase D/E: two t-half passes, each all-reduced
                with ExitStack() as dctx:
                    decs = dctx.enter_context(tc.tile_pool(name="decs",
                                                           bufs=2))
                    xspr = dctx.enter_context(tc.tile_pool(name="xspr",
                                                           bufs=3))
                    mpool = dctx.enter_context(tc.tile_pool(name="mpool",
                                                            bufs=3))
                    ysps = dctx.enter_context(
                        tc.tile_pool(name="ysps", bufs=2, space="PSUM"))
                    empool = dctx.enter_context(
                        tc.tile_pool(name="empool", bufs=4, space="PSUM"))
                    stp = dctx.enter_context(
                        tc.tile_pool(name="stp", bufs=2, space="PSUM"))
                    stageTT = sttp.tile([128, TBLK, D], BF16, name="stageTT",
                                        tag="stageTT")
                    DCH = 8  # dec ntl tiles per streamed chunk

                    for th in range(2):
                        tsl = slice(th * TH, (th + 1) * TH)
                        ymlp_th = [
                            empool.tile([128, TH], F32,
                                        name=f"ymlp_{th}{dh}", tag="ymlp_dt")
                            for dh in range(2)
                        ]
                        for ntl in range(NTILES):
                            if ntl % DCH == 0:
                                dec_c = decs.tile([128, DCH, D], BF16,
                                                  name="dec_c", tag="dec_c")
                                nc.sync.dma_start(
                                    out=dec_c,
                                    in_=dec_r[:, ntl:ntl + DCH, :])
                            if ntl % 2 == 0:
                                xspt = xspr.tile([128, 2, TH], BF16,
                                                 name="xspt", tag="xspt")
                                nc.sync.dma_start(
                                    out=xspt,
                                    in_=xsp_pair[:, ntl // 2][:, :, tsl])
                            ys = ysps.tile([128, TH], F32, name="ys_ps",
                                           tag="ys_ps")
                            nseg = slice(ntl * 128, (ntl + 1) * 128)
                            nc.tensor.matmul(
                                ys, lhsT=encv_s[:, 0, nseg],
                                rhs=yKVt[:, 0, tsl], start=True, stop=False)
                            nc.tensor.matmul(
                                ys, lhsT=encv_s[:, 1, nseg],
                                rhs=yKVt[:, 1, tsl], start=False, stop=True)
                            m_t = mpool.tile([128, TH], BF16, name="m_t",
                                             tag="m_t")
                            nc.scalar.activation(out=m_t, in_=ys,
                                                 func=AF.Relu)
                            nc.vector.tensor_mul(m_t, m_t,
                                                 xspt[:, ntl % 2, :])
                            for dh in range(2):
                                nc.tensor.matmul(
                                    ymlp_th[dh],
                                    lhsT=dec_c[:, ntl % DCH,
                                               dh * 128:(dh + 1) * 128],
                                    rhs=m_t,
                                    start=(ntl == 0),
                                    stop=(ntl == NTILES - 1),
                                    skip_group_check=True)
                        # transpose-stage this half in [T, D] layout + CC
                        sth = sthp.tile([128, 2, TH], BF16, name="sth",
                                        tag="sth")
                        for dh in range(2):
                            nc.scalar.copy(out=sth[:, dh, :],
                                           in_=ymlp_th[dh])
                        for q in range(4):
                            for dh in range(2):
                                tp = stp.tile([128, 128], BF16, name="stpt",
                                              tag="stpt")
                                nc.tensor.transpose(
                                    tp, sth[:, dh, q * 128:(q + 1) * 128],
                                    ident16)
                                nc.scalar.copy(
                                    out=stageTT[:, th * 4 + q,
                                                dh * 128:(dh + 1) * 128],
                                    in_=tp)
                        nc.sync.dma_start(
                            out=cc_in_r[:, th * 4:th * 4 + 4, :],
                            in_=stageTT[:, th * 4:th * 4 + 4, :])
                        if ABLATE_CC:
                            nc.sync.dma_start(out=cc_out_d[tsl, :],
                                              in_=cc_in_d[tsl, :])
                        else:
                            nc.gpsimd.collective_compute(
                                "AllReduce", ALU.add,
                                replica_groups=REPLICA_GROUPS,
                                ins=[cc_in_d[tsl, :]],
                                outs=[cc_out_d[tsl, :]])

        # ---------------- final layer's F halves + logits
        with tc.tile_pool(name="lps", bufs=2, space="PSUM") as lps, \
                tc.tile_pool(name="lout", bufs=2) as lout, \
                tc.tile_pool(name="lftp", bufs=2, space="PSUM") as lftp:
            x_bf = xbfp.tile([128, TBLK, D], BF16, name="x_bf", tag="x_bf")
            xT = xtp.tile([128, 2, T], BF16, name="xT", tag="xT")
            for th in range(2):
                emit_f_half(th, x_bf, xT, lftp, logits_pools=(lps, lout))

    _install_json_fix(nc)
    return nc


_NC_CACHE = {}


def _get_nc(n_layers=N_LAYER):
    if n_layers not in _NC_CACHE:
        _NC_CACHE[n_layers] = build_nc(n_layers)
    return _NC_CACHE[n_layers]


def prepare_in_maps(idx, encoder, encoder_v, decoder, embed, lm_head):
    idx = np.asarray(idx)
    encoder = np.asarray(encoder, dtype=np.float32)
    encoder_v = np.asarray(encoder_v, dtype=np.float32)
    decoder = np.asarray(decoder, dtype=np.float32)
    embed = np.asarray(embed, dtype=np.float32)
    lm_head = np.asarray(lm_head, dtype=np.float32)

    perm = _pair_perm()
    ropetab = _tables()
    lmh_bf = lm_head.astype(ml_dtypes.bfloat16)

    x0 = _ln_np(embed[idx])  # (B, T, D) f32
    dec3 = decoder.reshape(NH, N, D)

    in_maps = []
    for core in range(N_CORES):
        b, h = core // NH, core % NH
        in_maps.append({
            "x0": np.ascontiguousarray(x0[b]),
            "enc": np.ascontiguousarray(
                encoder[h][:, perm]).astype(ml_dtypes.bfloat16),
            "encv": np.ascontiguousarray(
                encoder_v[h][:, perm]).astype(ml_dtypes.bfloat16),
            "dec": np.ascontiguousarray(
                dec3[h][perm, :]).astype(ml_dtypes.bfloat16),
            "ropetab": ropetab,
            "lmh": lmh_bf,
        })
    return in_maps


def kernel(idx, encoder, encoder_v, decoder, embed, lm_head, *,
           trace=False, n_layers=N_LAYER):
    nc = _get_nc(n_layers)
    in_maps = prepare_in_maps(idx, encoder, encoder_v, decoder, embed, lm_head)
    res = run_bass_kernel_spmd(nc, in_maps, core_ids=list(range(N_CORES)),
                               trace=trace)
    out = np.stack([res.results[0]["logits"], res.results[NH]["logits"]])
    kernel.last_result = res
    return out.astype(np.float32)


# revision 30
# speedup vs baseline: 12.1637x; 12.1637x over previous
"""Trainium2 Bass kernel for an 8-layer weight-shared dense transformer variant.

Sharding: data-parallel over batch B=2 x tensor-parallel over the NH=4 heads
(core = 4*b + h). Each core runs the full layer stack for its (b, h) pair;
the yMLP head-contributions are summed with AllReduces over the 4 cores of
each batch group per layer.

v5 structure (per layer) - numerics identical to v2, pipelined collectives:
  - enc and encv are RESIDENT in SBUF bf16 (loaded once; the layer stack
    shares weights, so per-layer weight streaming was pure waste).  dec
    streams in batched [128,8,256] chunks per pass.
  - Phase D/E is split into two T-half passes; each half's yMLP is
    transpose-staged ([T,D] layout via PE transposes, so the collective
    needs no DMA transposes) and all-reduced separately.  CC(half0)
    overlaps the half1 compute pass; CC(half1) overlaps the next layer's
    first phase-A half-sweep, whose program order is interleaved with the
    previous layer's phase-F halves so no engine queue head-blocks.
  - Phase A runs as two t-half sweeps (matmul pair -> relu -> rope ->
    QRT half / spill half); the wide B1 score strips accumulate inside
    sweep 1 with a pair-group lag.  QR is fp8e4 x16, scores run DoubleRow
    fp8 trimmed to the causal triangle, rescaled 1/256 on the psum copy.
  - Narrow B2 strips + yKV tiles are interleaved so scores, LN chains and
    yKVt PE-transposes pipeline; phase D starts as soon as the first
    half of yKVt exists.
  - x_sparse spills to DRAM bf16 in [128,2,512] half DMAs and reloads
    pairwise per half in the D passes.
"""

import math
import os

import ml_dtypes
import numpy as np

import concourse.bass as bass
import concourse.mybir as mybir
import concourse.tile as tile
from concourse.bass_utils import run_bass_kernel_spmd
from concourse.masks import make_causal_mask, make_identity

BF16 = mybir.dt.bfloat16
F32 = mybir.dt.float32
FP8 = mybir.dt.float8e4
AF = mybir.ActivationFunctionType
ALU = mybir.AluOpType
DR = mybir.MatmulPerfMode.DoubleRow

N_CORES = 8
B, T, D, NH, N, VOCAB = 2, 1024, 256, 4, 8192, 256
NPAIR = N // 2
NTILES = N // 128  # 64
NGROUP = NPAIR // 128  # 32 pair groups (= packed k-tiles for DoubleRow)
TBLK = T // 128  # 8
TH = T // 2  # 512: the t-half width everything pipelines on
EPS = 1e-5
THETA = 2.0 ** 16
QSCALE = 16.0  # rope tables premultiplied by this; scores rescaled by 1/256
N_LAYER = int(os.environ.get("KERNEL_N_LAYER", "8"))
ABLATE_CC = os.environ.get("KERNEL_ABLATE_CC", "0") == "1"

REPLICA_GROUPS = [[0, 1, 2, 3], [4, 5, 6, 7]]

B1_CHUNKS = [(0, 0, 512), (0, 512, 512),
             (1, 128, 512), (1, 640, 384),
             (2, 256, 512)]
B2_CHUNKS = [(2, 768, 256),
             (3, 384, 512), (3, 896, 128),
             (4, 512, 512), (5, 640, 384),
             (6, 768, 256), (7, 896, 128)]
_SC_BASE = [0, 8, 15, 21, 26, 30, 33, 35]


def _sc_idx(sb, tb):
    return _SC_BASE[sb] + (tb - sb)


N_SC = 36


# ---------------------------------------------------------------- host side

def _split_multiwaits_json(bir: bytes) -> bytes:
    """This walrus build rejects instructions carrying more than one sync-wait
    ("Too many sync wait commands"), while Tile freely attaches several.
    Split: hoist all but the last wait of each instruction onto NoOps inserted
    immediately before it (same engine => executes right before it, so the
    AND-of-waits semantics is preserved)."""
    import json

    m = json.loads(bir)
    for func in m["functions"]:
        for blk in func["blocks"]:
            insts = blk["instructions"]
            out = []
            for inst in insts:
                si = inst.get("sync_info")
                waits = (si or {}).get("on_wait") or []
                if len(waits) > 1:
                    for k, w in enumerate(waits[:-1]):
                        out.append({
                            "engine": inst["engine"],
                            "ins": [],
                            "name": f"hw_{inst['name']}_{k}",
                            "opcode": "NoOp",
                            "outs": [],
                            "sync_info": {"on_update": [], "on_wait": [w]},
                        })
                    si["on_wait"] = [waits[-1]]
                out.append(inst)
            blk["instructions"] = out
    return json.dumps(m).encode()


def _install_json_fix(nc):
    orig = nc.to_json_bytes

    def patched():
        return _split_multiwaits_json(orig())

    nc.to_json_bytes = patched


def _pair_perm():
    """new index k -> original n.  Group j (256 original neurons) becomes an
    'even' tile (rows 2r) followed by an 'odd' tile (rows 2r+1)."""
    perm = np.empty(N, dtype=np.int64)
    r = np.arange(128)
    for j in range(NGROUP):
        base = 256 * j
        perm[base: base + 128] = base + 2 * r
        perm[base + 128: base + 256] = base + 2 * r + 1
    return perm


def _tables():
    """Rope table [NGROUP, 128, 2, T] bf16 = QSCALE * [c, -s] per pair row
    block.  With xsp2 = [e, o]:
      prods01 = xsp2 * tab        = [e*c, -o*s]  -> even tile = p0 + p1
      p2 = e * tab[1] = -e*s ; p3 = o * tab[0] = o*c -> odd = p3 - p2."""
    p = np.arange(NPAIR, dtype=np.float64)
    f = 1.0 / (THETA ** (2.0 * p / N)) / (2.0 * math.pi)
    t = np.arange(T, dtype=np.float64)
    ang = 2.0 * math.pi * np.mod(t[None, :] * f[:, None], 1.0)
    c = (QSCALE * np.cos(ang)).reshape(NGROUP, 128, T)
    s = (QSCALE * np.sin(ang)).reshape(NGROUP, 128, T)
    tab = np.stack([c, -s], axis=2)  # [NGROUP, 128, 2, T]
    return np.ascontiguousarray(tab).astype(ml_dtypes.bfloat16)


def _ln_np(x):
    x = x.astype(np.float64)
    mu = x.mean(-1, keepdims=True)
    var = ((x - mu) ** 2).mean(-1, keepdims=True)
    return ((x - mu) / np.sqrt(var + EPS)).astype(np.float32)


# ---------------------------------------------------------------- bass build

def build_nc(n_layers=N_LAYER):
    nc = bass.Bass("TRN2", target_bir_lowering=False, debug=False,
                   num_devices=N_CORES)

    x0_d = nc.dram_tensor("x0", [T, D], F32, kind="ExternalInput")
    enc_d = nc.dram_tensor("enc", [D, N], BF16, kind="ExternalInput")
    encv_d = nc.dram_tensor("encv", [D, N], BF16, kind="ExternalInput")
    dec_d = nc.dram_tensor("dec", [N, D], BF16, kind="ExternalInput")
    tab_d = nc.dram_tensor("ropetab", [NGROUP, 128, 2, T], BF16,
                           kind="ExternalInput")
    lmh_d = nc.dram_tensor("lmh", [D, VOCAB], BF16, kind="ExternalInput")
    out_d = nc.dram_tensor("logits", [T, VOCAB], F32, kind="ExternalOutput")
    xsp_d = nc.dram_tensor("xsp_scratch", [NTILES, 128, T], BF16).ap()
    cc_in_d = nc.dram_tensor("cc_in", [T, D], BF16).ap()
    cc_out_d = nc.dram_tensor("cc_out", [T, D], BF16).ap()
    warm_in = nc.dram_tensor("warm_in", [1, 64], BF16).ap()
    warm_out = nc.dram_tensor("warm_out", [1, 64], BF16).ap()

    enc_r = enc_d.ap().rearrange("(dh p) n -> p dh n", p=128)
    encv_r = encv_d.ap().rearrange("(dh p) n -> p dh n", p=128)
    dec_r = dec_d.ap().rearrange("(nt p) d -> p nt d", p=128)
    x0_r = x0_d.ap().rearrange("(i p) d -> p i d", p=128)
    lmh_r = lmh_d.ap().rearrange("(dh p) v -> p dh v", p=128)
    out_r = out_d.ap().rearrange("(i p) v -> p i v", p=128)
    tab_r = tab_d.ap()
    # batched spill view: pair pr covers tiles (2pr, 2pr+1) as [128, 2, T]
    xsp_pair = xsp_d.rearrange("(pr q) p t -> p pr q t", q=2)
    cc_in_r = cc_in_d.rearrange("(i p) d -> p i d", p=128)
    cc_out_r = cc_out_d.rearrange("(i p) d -> p i d", p=128)

    from contextlib import ExitStack

    with tile.TileContext(nc) as tc, ExitStack() as ctx:
        const = ctx.enter_context(tc.tile_pool(name="const", bufs=1))
        xpool = ctx.enter_context(tc.tile_pool(name="xpool", bufs=1))
        xbfp = ctx.enter_context(tc.tile_pool(name="xbfp", bufs=1))
        xtp = ctx.enter_context(tc.tile_pool(name="xtp", bufs=1))
        sthp = ctx.enter_context(tc.tile_pool(name="sthp", bufs=2))
        sttp = ctx.enter_context(tc.tile_pool(name="sttp", bufs=1))
        lnp = ctx.enter_context(tc.tile_pool(name="lnp", bufs=2))
        stats = ctx.enter_context(tc.tile_pool(name="stats", bufs=6))
        # persistent so next layer's tab loads overlap this layer's tail
        tabs = ctx.enter_context(tc.tile_pool(name="tabs", bufs=3))

        trim = const.tile([128, 128], F32, name="trim")
        make_causal_mask(nc, trim, mask_val=1.0)  # 1.0 where t > s else 0.0
        ident16 = const.tile([128, 128], BF16, name="ident16")
        make_identity(nc, ident16)
        lmh_t = const.tile([128, 2, VOCAB], BF16, name="lmh_t")
        nc.sync.dma_start(out=lmh_t, in_=lmh_r)
        eps_t = const.tile([128, 1], F32, name="eps_t")
        nc.vector.memset(eps_t, EPS)

        # resident bf16 weights, loaded once (shared across all layers);
        # loads are emitted after the x0 forms below and chunked so layer 0's
        # first matmuls don't wait on the full 8MB behind a serial DMA queue
        enc_s = const.tile([128, 2, N], BF16, name="enc_s")
        encv_s = const.tile([128, 2, N], BF16, name="encv_s")

        def layer_norm_tile(dst, src):
            """LN over the contiguous free (256) dim of one (128, 256) tile."""
            st = stats.tile([128, 6], F32, name="bn_st", tag="bn_st")
            mv = stats.tile([128, 2], F32, name="bn_mv", tag="bn_mv")
            rs = stats.tile([128, 1], F32, name="rstd", tag="rstd")
            nc.vector.bn_stats(out=st, in_=src)
            nc.vector.bn_aggr(out=mv, in_=st)
            nc.scalar.activation(out=rs, in_=mv[:, 1:2], func=AF.Sqrt,
                                 bias=eps_t, scale=1.0)
            nc.vector.reciprocal(rs, rs)
            nc.vector.tensor_scalar(
                out=dst, in0=src, scalar1=mv[:, 0:1], scalar2=rs,
                op0=ALU.subtract, op1=ALU.mult,
            )

        def make_x_forms(x_f32, tpp):
            """Initial x_bf [t,d] bf16 copy + xT [d,2,T] via PE transposes
            (DMA xbar transposes serialize against collectives in Tile)."""
            x_bf = xbfp.tile([128, TBLK, D], BF16, name="x_bf", tag="x_bf")
            xT = xtp.tile([128, 2, T], BF16, name="xT", tag="xT")
            for i in range(TBLK):
                nc.scalar.copy(out=x_bf[:, i, :], in_=x_f32[:, i, :])
                for dh in range(2):
                    tp = tpp.tile([128, 128], BF16, name="mxf", tag="mxf")
                    nc.tensor.transpose(
                        tp, x_bf[:, i, dh * 128:(dh + 1) * 128], ident16)
                    nc.scalar.copy(
                        out=xT[:, dh, i * 128:(i + 1) * 128], in_=tp)
            return x_bf, xT

        # initial x forms
        x_f32 = xpool.tile([128, TBLK, D], F32, name="x", tag="x")
        nc.sync.dma_start(out=x_f32, in_=x0_r)
        with tc.tile_pool(name="mxfp", bufs=2, space="PSUM") as mxfp:
            x_bf, xT = make_x_forms(x_f32, mxfp)
        for ch in range(4):
            nc.sync.dma_start(out=enc_s[:, :, ch * 2048:(ch + 1) * 2048],
                              in_=enc_r[:, :, ch * 2048:(ch + 1) * 2048])
        # encv is first used in layer 0's phase D; loading it there keeps
        # this 4MB transfer off the startup DMA queue ahead of the first
        # rope-table loads
        # warm up the collective path so layer 0's all-reduce doesn't pay
        # the one-time comm-init cost; emitted after the setup DMAs so
        # nothing at startup serializes behind it
        if not ABLATE_CC:
            nc.gpsimd.collective_compute(
                "AllReduce", ALU.add, replica_groups=REPLICA_GROUPS,
                ins=[warm_in], outs=[warm_out])

        def load_yln(th, pin=False):
            """Load t-half th of the cc output.  pin=True allocates from the
            tabs pool so the cc-gated DMA picks up a WAR dep on a late
            sweep-0 tab tile - otherwise the scheduler hoists it to the SP
            queue head, where its cc wait blocks every later DMA."""
            if pin:
                yln = tabs.tile([128, 4, D], BF16, name=f"yln{th}",
                                tag="tabh")
            else:
                yln = sthp.tile([128, 4, D], BF16, name=f"yln{th}",
                                tag="sth")
            nc.sync.dma_start(out=yln, in_=cc_out_r[:, th * 4:th * 4 + 4, :])
            return yln

        def emit_f_tile(th, q, yln, x_bf, xT, ftp, logits_pools=None):
            """One t-tile of phase F: LN + residual update (in place on
            x_f32) + x_bf/xT forms via PE transposes (DMA xbar transposes
            serialize against collectives in Tile)."""
            i = th * 4 + q
            ln_in = lnp.tile([128, D], F32, name="ln_in", tag="ln_in")
            layer_norm_tile(ln_in, yln[:, q, :])
            nc.vector.tensor_add(x_f32[:, i, :], x_f32[:, i, :], ln_in)
            layer_norm_tile(x_f32[:, i, :], x_f32[:, i, :])
            nc.scalar.copy(out=x_bf[:, i, :], in_=x_f32[:, i, :])
            for dh in range(2):
                tp = ftp.tile([128, 128], BF16, name="fxt", tag="fxt")
                nc.tensor.transpose(
                    tp, x_bf[:, i, dh * 128:(dh + 1) * 128], ident16)
                nc.scalar.copy(
                    out=xT[:, dh, i * 128:(i + 1) * 128], in_=tp)
            if logits_pools is not None:
                lps, lout = logits_pools
                ps = lps.tile([128, VOCAB], F32, name="l_ps", tag="l_ps")
                nc.tensor.matmul(
                    ps, lhsT=xT[:, 0, i * 128:(i + 1) * 128],
                    rhs=lmh_t[:, 0, :], start=True, stop=False)
                nc.tensor.matmul(
                    ps, lhsT=xT[:, 1, i * 128:(i + 1) * 128],
                    rhs=lmh_t[:, 1, :], start=False, stop=True)
                ot = lout.tile([128, VOCAB], F32, name="l_sb", tag="l_sb")
                nc.scalar.copy(out=ot, in_=ps)
                nc.sync.dma_start(out=out_r[:, i, :], in_=ot)

        def emit_f_half(th, x_bf, xT, ftp, logits_pools=None, pin=False):
            yln = load_yln(th, pin=pin)
            for q in range(4):
                emit_f_tile(th, q, yln, x_bf, xT, ftp, logits_pools)

        for li in range(n_layers):
            if li > 0:
                x_bf = xbfp.tile([128, TBLK, D], BF16, name="x_bf",
                                 tag="x_bf")
                xT = xtp.tile([128, 2, T], BF16, name="xT", tag="xT")
            with ExitStack() as lctx:
                qrt_p = lctx.enter_context(tc.tile_pool(name="qrt", bufs=1))
                scp = lctx.enter_context(tc.tile_pool(name="scp", bufs=1))
                QRT = qrt_p.tile([128, NTILES, T], FP8, name="QRT", tag="QRT")
                scb = scp.tile([128, N_SC, 128], BF16, name="scb", tag="scb")

                # -------- phase A: two t-half sweeps (+ prev layer's F
                # halves interleaved so CC(half1) hides under sweep 0)
                with ExitStack() as actx:
                    xsps = actx.enter_context(tc.tile_pool(name="xsps",
                                                           bufs=3))
                    ropet = actx.enter_context(tc.tile_pool(name="ropet",
                                                            bufs=2))
                    apsum = actx.enter_context(
                        tc.tile_pool(name="apsum", bufs=2, space="PSUM"))
                    ftp = actx.enter_context(
                        tc.tile_pool(name="ftp", bufs=1, space="PSUM"))
                    b1psum = actx.enter_context(
                        tc.tile_pool(name="b1psum", bufs=1, space="PSUM"))
                    b1_ps = {
                        ch: b1psum.tile([128, ch[2]], F32, name=f"b1_{i}",
                                        tag=f"b1_{i}")
                        for i, ch in enumerate(B1_CHUNKS)
                    }
                    LAG = 2

                    def b1_step(kt):
                        for ch in B1_CHUNKS:
                            sb, t0, w = ch
                            nc.tensor.matmul(
                                b1_ps[ch],
                                lhsT=QRT[:, 2 * kt:2 * kt + 2,
                                         sb * 128:(sb + 1) * 128],
                                rhs=QRT[:, 2 * kt:2 * kt + 2, t0:t0 + w],
                                start=(kt == 0), stop=(kt == NGROUP - 1),
                                perf_mode=DR, skip_group_check=True)

                    def sweep(th):
                        tsl = slice(th * TH, (th + 1) * TH)
                        for j in range(NGROUP):
                            tabh = tabs.tile([128, 2, TH], BF16, name="tabh",
                                             tag="tabh")
                            nc.sync.dma_start(out=tabh,
                                              in_=tab_r[j][:, :, tsl])
                            xsp2 = xsps.tile([128, 2, TH], BF16, name="xsp2",
                                             tag="xsp2")
                            prods = ropet.tile([128, 4, TH], BF16,
                                               name="prods", tag="prods")
                            for par in range(2):
                                nseg = slice(256 * j + 128 * par,
                                             256 * j + 128 * par + 128)
                                ps = apsum.tile([128, TH], F32, name="aps",
                                                tag="aps")
                                nc.tensor.matmul(
                                    ps, lhsT=enc_s[:, 0, nseg],
                                    rhs=xT[:, 0, tsl],
                                    start=True, stop=False)
                                nc.tensor.matmul(
                                    ps, lhsT=enc_s[:, 1, nseg],
                                    rhs=xT[:, 1, tsl],
                                    start=False, stop=True)
                                nc.scalar.activation(
                                    out=xsp2[:, par, :], in_=ps,
                                    func=AF.Relu)
                                # prods = [e*c, -e*s, o*c, -o*s]
                                nc.vector.tensor_mul(
                                    prods[:, 2 * par, :], xsp2[:, par, :],
                                    tabh[:, 0, :])
                                nc.vector.tensor_mul(
                                    prods[:, 2 * par + 1, :],
                                    xsp2[:, par, :], tabh[:, 1, :])
                            nc.sync.dma_start(
                                out=xsp_pair[:, j][:, :, tsl], in_=xsp2)
                            # even = e*c + o*(-s); odd = o*c + e*s
                            nc.vector.tensor_add(QRT[:, 2 * j, tsl],
                                                 prods[:, 0, :],
                                                 prods[:, 3, :])
                            nc.gpsimd.tensor_sub(QRT[:, 2 * j + 1, tsl],
                                                 prods[:, 2, :],
                                                 prods[:, 1, :])
                            if th == 1 and j >= LAG:
                                b1_step(j - LAG)
                            if hook is not None:
                                hook(j)

                    if li > 0:
                        emit_f_half(0, x_bf, xT, ftp)
                        # F half 1 interleaves into sweep 0's tail: cc1 is
                        # long done by group 26, so its LN/transpose chains
                        # execute as the last rope groups drain instead of
                        # serializing after the whole sweep
                        f1_state = {}

                        def f1_hook(j):
                            if j == NGROUP - 6:
                                f1_state["yln"] = load_yln(1, pin=True)
                            elif j >= NGROUP - 5 and j <= NGROUP - 2:
                                emit_f_tile(1, j - (NGROUP - 5),
                                            f1_state["yln"], x_bf, xT, ftp)
                    else:
                        f1_hook = None
                    hook = f1_hook
                    sweep(0)
                    hook = None
                    sweep(1)
                    for kt in range(NGROUP - LAG, NGROUP):
                        b1_step(kt)
                    # copy B1 strips to sc blocks (scaled 1/256, diag trimmed)
                    for ch in B1_CHUNKS:
                        sb, t0, w = ch
                        for q in range(w // 128):
                            tb = t0 // 128 + q
                            dst = scb[:, _sc_idx(sb, tb), :]
                            src = b1_ps[ch][:, q * 128:(q + 1) * 128]
                            if tb == sb:
                                nc.vector.scalar_tensor_tensor(
                                    out=dst, in0=src, scalar=1.0 / 256.0,
                                    in1=trim, op0=ALU.mult, op1=ALU.mult)
                            else:
                                nc.scalar.activation(out=dst, in_=src,
                                                     func=AF.Copy,
                                                     scale=1.0 / 256.0)

                # ---------------- phase B2 + C interleaved
                with ExitStack() as bctx:
                    b2psum = bctx.enter_context(
                        tc.tile_pool(name="b2psum", bufs=2, space="PSUM"))
                    ykvln = lctx.enter_context(
                        tc.tile_pool(name="ykvln", bufs=1))
                    ykvq = lctx.enter_context(
                        tc.tile_pool(name="ykvq", bufs=2))
                    yKVt = ykvln.tile([128, 2, T], BF16, name="yKVt",
                                      tag="yKVt")
                    ykvps = bctx.enter_context(
                        tc.tile_pool(name="ykvps", bufs=2, space="PSUM"))
                    ctp = bctx.enter_context(
                        tc.tile_pool(name="ctp", bufs=2, space="PSUM"))

                    def b2_chunk(i):
                        sb, t0, w = B2_CHUNKS[i]
                        ps = b2psum.tile([128, w], F32, name=f"b2_{i}",
                                         tag="b2")
                        for j in range(NGROUP):
                            nc.tensor.matmul(
                                ps,
                                lhsT=QRT[:, 2 * j:2 * j + 2,
                                         sb * 128:(sb + 1) * 128],
                                rhs=QRT[:, 2 * j:2 * j + 2, t0:t0 + w],
                                start=(j == 0), stop=(j == NGROUP - 1),
                                perf_mode=DR)
                        for q in range(w // 128):
                            tb = t0 // 128 + q
                            dst = scb[:, _sc_idx(sb, tb), :]
                            srcp = ps[:, q * 128:(q + 1) * 128]
                            if tb == sb:
                                nc.vector.scalar_tensor_tensor(
                                    out=dst, in0=srcp, scalar=1.0 / 256.0,
                                    in1=trim, op0=ALU.mult, op1=ALU.mult)
                            else:
                                nc.scalar.activation(out=dst, in_=srcp,
                                                     func=AF.Copy,
                                                     scale=1.0 / 256.0)

                    def ykv_tile(tb):
                        yk = ykvps.tile([128, D], F32, name="yk", tag="yk")
                        for sb in range(tb + 1):
                            nc.tensor.matmul(
                                yk,
                                lhsT=scb[:, _sc_idx(sb, tb), :],
                                rhs=x_bf[:, sb, :],
                                start=(sb == 0), stop=(sb == tb))
                        ykv_td = ykvq.tile([128, D], BF16, name="ykv_td",
                                           tag="ykv_td")
                        layer_norm_tile(ykv_td, yk)
                        for dh in range(2):
                            tp = ctp.tile([128, 128], BF16, name="ctpt",
                                          tag="ctpt")
                            nc.tensor.transpose(
                                tp, ykv_td[:, dh * 128:(dh + 1) * 128],
                                ident16)
                            nc.scalar.copy(
                                out=yKVt[:, dh, tb * 128:(tb + 1) * 128],
                                in_=tp)

                    b2_after = {2: 1, 3: 3, 4: 4, 5: 5, 6: 6, 7: 7}
                    emitted = 0
                    for tb in range(TBLK):
                        while emitted < b2_after.get(tb, 0):
                            b2_chunk(emitted)
                            emitted += 1
                        ykv_tile(tb)
                    while emitted < len(B2_CHUNKS):
                        b2_chunk(emitted)
                        emitted += 1

                # -------- phase D/E: two t-half passes, each all-reduced
                if li == 0:
                    nc.sync.dma_start(out=encv_s, in_=encv_r)
                with ExitStack() as dctx:
                    decs = dctx.enter_context(tc.tile_pool(name="decs",
                                                           bufs=2))
                    xspr = dctx.enter_context(tc.tile_pool(name="xspr",
                                                           bufs=3))
                    mpool = dctx.enter_context(tc.tile_pool(name="mpool",
                                                            bufs=3))
                    ysps = dctx.enter_context(
                        tc.tile_pool(name="ysps", bufs=2, space="PSUM"))
                    empool = dctx.enter_context(
                        tc.tile_pool(name="empool", bufs=4, space="PSUM"))
                    stp = dctx.enter_context(
                        tc.tile_pool(name="stp", bufs=2, space="PSUM"))
                    stageTT = sttp.tile([128, TBLK, D], BF16, name="stageTT",
                                        tag="stageTT")
                    DCH = 8  # dec ntl tiles per streamed chunk

                    for th in range(2):
                        tsl = slice(th * TH, (th + 1) * TH)
                        ymlp_th = [
                            empool.tile([128, TH], F32,
                                        name=f"ymlp_{th}{dh}", tag="ymlp_dt")
                            for dh in range(2)
                        ]
                        for ntl in range(NTILES):
                            if ntl % DCH == 0:
                                dec_c = decs.tile([128, DCH, D], BF16,
                                                  name="dec_c", tag="dec_c")
                                nc.sync.dma_start(
                                    out=dec_c,
                                    in_=dec_r[:, ntl:ntl + DCH, :])
                            if ntl % 2 == 0:
                                xspt = xspr.tile([128, 2, TH], BF16,
                                                 name="xspt", tag="xspt")
                                nc.sync.dma_start(
                                    out=xspt,
                                    in_=xsp_pair[:, ntl // 2][:, :, tsl])
                            ys = ysps.tile([128, TH], F32, name="ys_ps",
                                           tag="ys_ps")
                            nseg = slice(ntl * 128, (ntl + 1) * 128)
                            nc.tensor.matmul(
                                ys, lhsT=encv_s[:, 0, nseg],
                                rhs=yKVt[:, 0, tsl], start=True, stop=False)
                            nc.tensor.matmul(
                                ys, lhsT=encv_s[:, 1, nseg],
                                rhs=yKVt[:, 1, tsl], start=False, stop=True)
                            m_t = mpool.tile([128, TH], BF16, name="m_t",
                                             tag="m_t")
                            nc.scalar.activation(out=m_t, in_=ys,
                                                 func=AF.Relu)
                            nc.vector.tensor_mul(m_t, m_t,
                                                 xspt[:, ntl % 2, :])
                            for dh in range(2):
                                nc.tensor.matmul(
                                    ymlp_th[dh],
                                    lhsT=dec_c[:, ntl % DCH,
                                               dh * 128:(dh + 1) * 128],
                                    rhs=m_t,
                                    start=(ntl == 0),
                                    stop=(ntl == NTILES - 1),
                                    skip_group_check=True)
                        # transpose-stage this half in [T, D] layout + CC
                        sth = sthp.tile([128, 2, TH], BF16, name="sth",
                                        tag="sth")
                        for dh in range(2):
                            nc.scalar.copy(out=sth[:, dh, :],
                                           in_=ymlp_th[dh])
                        for q in range(4):
                            for dh in range(2):
                                tp = stp.tile([128, 128], BF16, name="stpt",
                                              tag="stpt")
                                nc.tensor.transpose(
                                    tp, sth[:, dh, q * 128:(q + 1) * 128],
                                    ident16)
                                nc.scalar.copy(
                                    out=stageTT[:, th * 4 + q,
                                                dh * 128:(dh + 1) * 128],
                                    in_=tp)
                        nc.sync.dma_start(
                            out=cc_in_r[:, th * 4:th * 4 + 4, :],
                            in_=stageTT[:, th * 4:th * 4 + 4, :])
                        if ABLATE_CC:
                            nc.sync.dma_start(out=cc_out_d[tsl, :],
                                              in_=cc_in_d[tsl, :])
                        else:
                            nc.gpsimd.collective_compute(
                                "AllReduce", ALU.add,
                                replica_groups=REPLICA_GROUPS,
                                ins=[cc_in_d[tsl, :]],
                                outs=[cc_out_d[tsl, :]])

        # ---------------- final layer's F halves + logits
        with tc.tile_pool(name="lps", bufs=2, space="PSUM") as lps, \
                tc.tile_pool(name="lout", bufs=2) as lout, \
                tc.tile_pool(name="lftp", bufs=2, space="PSUM") as lftp:
            x_bf = xbfp.tile([128, TBLK, D], BF16, name="x_bf", tag="x_bf")
            xT = xtp.tile([128, 2, T], BF16, name="xT", tag="xT")
            for th in range(2):
                emit_f_half(th, x_bf, xT, lftp, logits_pools=(lps, lout))

    _install_json_fix(nc)
    return nc


_NC_CACHE = {}


def _get_nc(n_layers=N_LAYER):
    if n_layers not in _NC_CACHE:
        _NC_CACHE[n_layers] = build_nc(n_layers)
    return _NC_CACHE[n_layers]


def prepare_in_maps(idx, encoder, encoder_v, decoder, embed, lm_head):
    idx = np.asarray(idx)
    encoder = np.asarray(encoder, dtype=np.float32)
    encoder_v = np.asarray(encoder_v, dtype=np.float32)
    decoder = np.asarray(decoder, dtype=np.float32)
    embed = np.asarray(embed, dtype=np.float32)
    lm_head = np.asarray(lm_head, dtype=np.float32)

    perm = _pair_perm()
    ropetab = _tables()
    lmh_bf = lm_head.astype(ml_dtypes.bfloat16)

    x0 = _ln_np(embed[idx])  # (B, T, D) f32
    dec3 = decoder.reshape(NH, N, D)

    in_maps = []
    for core in range(N_CORES):
        b, h = core // NH, core % NH
        in_maps.append({
            "x0": np.ascontiguousarray(x0[b]),
            "enc": np.ascontiguousarray(
                encoder[h][:, perm]).astype(ml_dtypes.bfloat16),
            "encv": np.ascontiguousarray(
                encoder_v[h][:, perm]).astype(ml_dtypes.bfloat16),
            "dec": np.ascontiguousarray(
                dec3[h][perm, :]).astype(ml_dtypes.bfloat16),
            "ropetab": ropetab,
            "lmh": lmh_bf,
        })
    return in_maps


def kernel(idx, encoder, encoder_v, decoder, embed, lm_head, *,
           trace=False, n_layers=N_LAYER):
    nc = _get_nc(n_layers)
    in_maps = prepare_in_maps(idx, encoder, encoder_v, decoder, embed, lm_head)
    res = run_bass_kernel_spmd(nc, in_maps, core_ids=list(range(N_CORES)),
                               trace=trace)
    out = np.stack([res.results[0]["logits"], res.results[NH]["logits"]])
    kernel.last_result = res
    return out.astype(np.float32)
